# revision 10
# baseline (speedup 1.0000x reference)
"""Trainium2 Bass kernel for nn_AUFusion (dense_mlp, memory-bound).

Reference computation (per sample b):
  feat[b, c]   = sum_k act_c[b, k] * gcn[b, c, k]    act_c = eyebrow (c<3) / mouth (c>=3)
  normed       = LayerNorm(feat) * ln_w + ln_b       (over the 9 features, eps=1e-6)
  out[b, :]    = normed @ lin_w.T + lin_b            (9 -> 5)

Strategy: pure data parallelism, batch 16384 -> 2048 per core on 8 cores.
On-chip layout puts 128 samples on partitions and K=512 on the free axis.

The 16 DMA queues sustain ~27 GB/s each (432 GB/s aggregate); in fp32 the 44
MiB/core input stream has a hard ~107 us floor and DVE (144 dot products at
~0.69 us) needs ~105 us — both saturated. The host therefore downcasts the
streamed tensors to fp16 (inputs are ~N(0,1); the fp32 accumulation keeps
rel err ~1e-4, well under the 2e-2 gate), halving the stream to ~53 us and
making compute the only critical path. Each dot product is one fp16
scalar_tensor_tensor on DVE (out = (g*1)*a with fp32 accum_out, ~0.61 us);
LayerNorm + the (host-folded) LN-affine+Linear projection run batched over
4-tile chunks in DVE slack. Tile 0's gcn is split 3/3/3 so the first dots
ungate early; tile 15 is split 6/3 to shorten the drain. Output is stored
as [128, 16, 5] and transposed on host.
"""

import numpy as np

import concourse.bacc as bacc
import concourse.tile as tile
from concourse import mybir
from concourse.bass_utils import run_bass_kernel_spmd

N_CORES = 8
B = 16384
BPC = B // N_CORES          # samples per core
K = 512
C = 9                       # in features
NCLS = 5                    # num classes
P = 128                     # partitions
NT = BPC // P               # 16 sample-tiles per core
LN_EPS = 1e-6
F32 = mybir.dt.float32
F16 = mybir.dt.float16

_NC = None  # built once, reused across calls


def _build_nc():
    nc = bacc.Bacc(None)
    # host-pretransposed, interleaved act (fp16): a[p, t*2K + (0:K)] =
    # eyebrow[t*128+p, :], a[p, t*2K + (K:2K)] = mouth[t*128+p, :]
    act = nc.dram_tensor("act", [P, NT * 2 * K], F16, kind="ExternalInput")
    gcn = nc.dram_tensor("gcn", [BPC, C, K], F16, kind="ExternalInput")
    # merged consts: [w2 (NCLS*C) | b2 (NCLS)] broadcast over partitions
    wb = nc.dram_tensor("wb", [P, NCLS * C + NCLS], F32, kind="ExternalInput")
    # [p, t, j] layout; host transposes to [t*128+p, j]
    out = nc.dram_tensor("out", [P, NT * NCLS], F32, kind="ExternalOutput")

    mult = mybir.AluOpType.mult
    add = mybir.AluOpType.add

    with tile.TileContext(nc) as tc:
        with (
            tc.tile_pool(name="gcnp", bufs=6) as gcnp,
            tc.tile_pool(name="actp", bufs=6) as actp,
            tc.tile_pool(name="big", bufs=1) as big,
        ):
            feat = big.tile([P, NT * C], F32)
            dscr = big.tile([P, K], F16)   # discard target for STT full out
            wb_sb = big.tile([P, NCLS * C + NCLS], F32)
            w2_sb = wb_sb[:, :NCLS * C].rearrange("p (j c) -> p j c", c=C)
            b2_sb = wb_sb[:, NCLS * C:]
            y = big.tile([P, NT * NCLS], F32)
            dscr_a = big.tile([P, K], F16)  # ACT-engine discard out
            ps0 = big.tile([P, K], F16)
            ps1 = big.tile([P, K], F16)
            ps2 = big.tile([P, K], F16)
            ps3 = big.tile([P, K], F16)
            ps4 = big.tile([P, K], F16)
            ps5 = big.tile([P, K], F16)
            pscr = [ps0, ps1, ps2, ps3, ps4, ps5]
            g0a = big.tile([P, 3 * K], F16)    # tile-0 gcn rows c0..2
            g0b = big.tile([P, 6 * K], F16)    # tile-0 gcn rows c3..8
            g15a = big.tile([P, 6 * K], F16)   # tile-15 gcn rows c0..5
            g15b = big.tile([P, 3 * K], F16)   # tile-15 gcn rows c6..8

            def ln_proj(t0, ntl):
                """Batched LayerNorm + projection for tiles [t0, t0+ntl)."""
                f3 = feat[:, t0 * C:(t0 + ntl) * C].rearrange(
                    "p (t c) -> p t c", c=C
                )
                negmu = big.tile([P, ntl], F32, tag=f"negmu{t0}")
                nc.vector.tensor_reduce(
                    out=negmu[:], in_=f3, axis=mybir.AxisListType.X, op=add
                )
                nc.vector.tensor_scalar_mul(negmu[:], negmu[:], -1.0 / C)
                cent = big.tile([P, ntl * C], F32, tag=f"cent{t0}")
                c3 = cent[:].rearrange("p (t c) -> p t c", c=C)
                nc.vector.tensor_tensor(
                    c3, f3, negmu[:][:, :, None].to_broadcast([P, ntl, C]), op=add
                )
                sq = big.tile([P, ntl * C], F32, tag=f"sq{t0}")
                s3 = sq[:].rearrange("p (t c) -> p t c", c=C)
                nc.vector.tensor_tensor(s3, c3, c3, op=mult)
                varp = big.tile([P, ntl], F32, tag=f"varp{t0}")
                nc.vector.tensor_reduce(
                    out=varp[:], in_=s3, axis=mybir.AxisListType.X, op=add
                )
                nc.vector.tensor_scalar(
                    out=varp[:], in0=varp[:], scalar1=1.0 / C, scalar2=LN_EPS,
                    op0=mult, op1=add,
                )
                std = big.tile([P, ntl], F32, tag=f"std{t0}")
                nc.scalar.activation(
                    std[:], varp[:], mybir.ActivationFunctionType.Sqrt
                )
                rstd = big.tile([P, ntl], F32, tag=f"rstd{t0}")
                nc.vector.reciprocal(rstd[:], std[:])
                xhat = big.tile([P, ntl * C], F32, tag=f"xhat{t0}")
                x3 = xhat[:].rearrange("p (t c) -> p t c", c=C)
                nc.vector.tensor_tensor(
                    x3, c3, rstd[:][:, :, None].to_broadcast([P, ntl, C]), op=mult
                )
                prod = big.tile([P, ntl * NCLS * C], F32, tag=f"prod{t0}")
                p4 = prod[:].rearrange("p (t j c) -> p t j c", j=NCLS, c=C)
                nc.vector.tensor_tensor(
                    p4,
                    x3[:, :, None, :].to_broadcast([P, ntl, NCLS, C]),
                    w2_sb[:, None, :, :].to_broadcast([P, ntl, NCLS, C]),
                    op=mult,
                )
                y3 = y[:, t0 * NCLS:(t0 + ntl) * NCLS].rearrange(
                    "p (t j) -> p t j", j=NCLS
                )
                nc.vector.tensor_reduce(
                    out=y3, in_=p4, axis=mybir.AxisListType.X, op=add
                )
                nc.vector.tensor_tensor(
                    y3, y3, b2_sb[:, None, :].to_broadcast([P, ntl, NCLS]), op=add
                )

            def dot(accum_col, g_ap, a_ap):
                """One fp16 dot product on DVE, fp32 accumulate."""
                nc.vector.scalar_tensor_tensor(
                    out=dscr[:],
                    in0=g_ap,
                    scalar=1.0,
                    in1=a_ap,
                    op0=mult,
                    op1=mult,
                    accum_out=feat[:, accum_col:accum_col + 1],
                )

            def dot_off(accum_col, g_ap, a_ap, buf, on_gpsimd):
                eng = nc.gpsimd if on_gpsimd else nc.vector
                eng.tensor_tensor(buf[:], g_ap, a_ap, op=mult)
                nc.scalar.activation(
                    dscr_a[:], buf[:], mybir.ActivationFunctionType.Copy,
                    accum_out=feat[:, accum_col:accum_col + 1],
                )

            # ---- tile 0: split gcn 3/3/3 so first dots ungate early ----
            a0 = actp.tile([P, 2 * K], F16, tag="a")
            nc.sync.dma_start(a0[:], act[:, 0:2 * K])
            g0 = gcn[0:P]  # [128, 9, 512]
            nc.sync.dma_start(
                g0a[:].rearrange("p (c k) -> p c k", c=3), g0[:, 0:3, :]
            )
            nc.sync.dma_start(
                g0b[:, :3 * K].rearrange("p (c k) -> p c k", c=3), g0[:, 3:6, :]
            )
            nc.sync.dma_start(
                g0b[:, 3 * K:].rearrange("p (c k) -> p c k", c=3), g0[:, 6:9, :]
            )
            nc.scalar.dma_start(wb_sb[:], wb[:])
            ae, am = a0[:, 0:K], a0[:, K:2 * K]
            for c in range(3):
                dot(c, g0a[:, c * K:(c + 1) * K], ae)
            for c in range(3, 6):
                dot(c, g0b[:, (c - 3) * K:(c - 2) * K], am)
            dot_off(6, g0b[:, 3 * K:4 * K], am, pscr[0], False)
            dot_off(7, g0b[:, 4 * K:5 * K], am, pscr[1], True)
            dot_off(8, g0b[:, 5 * K:6 * K], am, pscr[2], True)

            # ---- tiles 1..14: streaming steady state ----
            for t in range(1, NT - 1):
                a_t = actp.tile([P, 2 * K], F16, tag="a")
                nc.sync.dma_start(a_t[:], act[:, t * 2 * K:(t + 1) * 2 * K])
                g_t = gcnp.tile([P, C * K], F16)
                nc.sync.dma_start(
                    g_t[:].rearrange("p (c k) -> p c k", c=C),
                    gcn[t * P:(t + 1) * P],
                )
                ae, am = a_t[:, 0:K], a_t[:, K:2 * K]
                for c in range(6):
                    dot(t * C + c, g_t[:, c * K:(c + 1) * K], ae if c < 3 else am)
                dot_off(t * C + 6, g_t[:, 6 * K:7 * K], am, pscr[t % 2], False)
                dot_off(t * C + 7, g_t[:, 7 * K:8 * K], am, pscr[2 + (2 * t) % 4], True)
                dot_off(t * C + 8, g_t[:, 8 * K:9 * K], am, pscr[2 + (2 * t + 1) % 4], True)
                # LN chunks slot into DVE slack (DMA has 2x headroom in fp16)
                if t == 5:
                    ln_proj(0, 4)
                elif t == 9:
                    ln_proj(4, 4)
                elif t == 13:
                    ln_proj(8, 4)
                elif t == 14:
                    ln_proj(12, 3)

            # ---- tile 15: split 6/3 to shorten the drain ----
            t = NT - 1
            a15 = actp.tile([P, 2 * K], F16, tag="a")
            nc.sync.dma_start(a15[:], act[:, t * 2 * K:(t + 1) * 2 * K])
            g15 = gcn[t * P:(t + 1) * P]
            nc.sync.dma_start(
                g15a[:].rearrange("p (c k) -> p c k", c=6), g15[:, 0:6, :]
            )
            nc.sync.dma_start(
                g15b[:].rearrange("p (c k) -> p c k", c=3), g15[:, 6:9, :]
            )
            # first output piece: y[0:12] is final after ln_proj(8, 4)
            nc.scalar.dma_start(out[:, :12 * NCLS], y[:, :12 * NCLS])
            ae, am = a15[:, 0:K], a15[:, K:2 * K]
            for c in range(6):
                dot(t * C + c, g15a[:, c * K:(c + 1) * K], ae if c < 3 else am)
            dot_off(t * C + 6, g15b[:, 0:K], am, pscr[0], False)
            dot_off(t * C + 7, g15b[:, K:2 * K], am, pscr[1], True)
            dot_off(t * C + 8, g15b[:, 2 * K:3 * K], am, pscr[2], True)
            ln_proj(15, 1)
            nc.scalar.dma_start(out[:, 12 * NCLS:], y[:, 12 * NCLS:])

    nc.finalize()
    return nc


def _get_nc():
    global _NC
    if _NC is None:
        _NC = _build_nc()
    return _NC


def _run(inputs, **spmd_kwargs):
    eyebrow = np.asarray(inputs["eyebrow"]).astype(np.float16)
    mouth = np.asarray(inputs["mouth"]).astype(np.float16)
    gcn = np.ascontiguousarray(np.asarray(inputs["gcn"]).astype(np.float16))
    ln_w = np.asarray(inputs["ln_weight"], dtype=np.float32)
    ln_b = np.asarray(inputs["ln_bias"], dtype=np.float32)
    lin_w = np.asarray(inputs["lin_weight"], dtype=np.float32)
    lin_b = np.asarray(inputs["lin_bias"], dtype=np.float32)

    # Fold LN affine + Linear: normed*ln_w + ln_b then @ lin_w.T + lin_b
    #   == xhat @ W2 + b2 with W2[c,j] = ln_w[c]*lin_w[j,c], b2 = lin_w@ln_b + lin_b
    w2 = (lin_w * ln_w[None, :]).astype(np.float32)        # [NCLS, C] = W2.T
    b2 = (lin_w @ ln_b + lin_b).astype(np.float32)         # [NCLS]
    wb1 = np.concatenate([w2.ravel(), b2]).astype(np.float32)
    wb = np.ascontiguousarray(np.broadcast_to(wb1[None], (P, NCLS * C + NCLS)))

    # per-core partition-major interleaved act layout: [P, NT, 2, K]
    a_sh = np.stack(
        [eyebrow.reshape(N_CORES, NT, P, K), mouth.reshape(N_CORES, NT, P, K)],
        axis=3,
    )  # [cores, NT, P, 2, K]
    a_sh = np.ascontiguousarray(a_sh.transpose(0, 2, 1, 3, 4)).reshape(
        N_CORES, P, NT * 2 * K
    )
    g_sh = gcn.reshape(N_CORES, BPC, C, K)
    in_maps = [
        {"act": a_sh[c], "gcn": g_sh[c], "wb": wb}
        for c in range(N_CORES)
    ]

    res = run_bass_kernel_spmd(
        _get_nc(), in_maps, core_ids=list(range(N_CORES)), **spmd_kwargs
    )
    # out[p, t*5+j] -> full[(core, t*128+p), j]
    out = np.concatenate(
        [
            r["out"].reshape(P, NT, NCLS).transpose(1, 0, 2).reshape(BPC, NCLS)
            for r in res.results
        ],
        axis=0,
    )
    return out, res


def kernel(**inputs):
    out, _ = _run(inputs)
    return out


# revision 11
# speedup vs baseline: 1.1457x; 1.1457x over previous
"""Trainium2 Bass kernel for nn_AUFusion (dense_mlp, memory-bound).

Reference computation (per sample b):
  feat[b, c]   = sum_k act_c[b, k] * gcn[b, c, k]    act_c = eyebrow (c<3) / mouth (c>=3)
  normed       = LayerNorm(feat) * ln_w + ln_b       (over the 9 features, eps=1e-6)
  out[b, :]    = normed @ lin_w.T + lin_b            (9 -> 5)

Strategy: pure data parallelism, batch 16384 -> 2048 per core on 8 cores.
On-chip layout puts 128 samples on partitions and K=512 on the free axis.

The 16 DMA queues sustain ~27 GB/s each (432 GB/s aggregate); in fp32 the 44
MiB/core input stream has a hard ~107 us floor and DVE (144 dot products at
~0.69 us) needs ~105 us — both saturated. The host therefore downcasts the
streamed tensors to fp16 (inputs are ~N(0,1); the fp32 accumulation keeps
rel err ~1e-4, well under the 2e-2 gate), halving the stream to ~53 us and
making compute the only critical path. Each dot product is one fp16
scalar_tensor_tensor on DVE (out = (g*1)*a with fp32 accum_out, ~0.61 us);
LayerNorm + the (host-folded) LN-affine+Linear projection run batched over
4-tile chunks in DVE slack. Tile 0's gcn is split 3/3/3 so the first dots
ungate early; tile 15 is split 6/3 to shorten the drain. Output is stored
as [128, 16, 5] and transposed on host.
"""

import numpy as np

import concourse.bacc as bacc
import concourse.tile as tile
from concourse import mybir
from concourse.bass_utils import run_bass_kernel_spmd

N_CORES = 8
B = 16384
BPC = B // N_CORES          # samples per core
K = 512
C = 9                       # in features
NCLS = 5                    # num classes
P = 128                     # partitions
NT = BPC // P               # 16 sample-tiles per core
LN_EPS = 1e-6
F32 = mybir.dt.float32
F16 = mybir.dt.float16

_NC = None  # built once, reused across calls


def _build_nc():
    nc = bacc.Bacc(None)
    # host-pretransposed, interleaved act (fp16): a[p, t*2K + (0:K)] =
    # eyebrow[t*128+p, :], a[p, t*2K + (K:2K)] = mouth[t*128+p, :]
    act = nc.dram_tensor("act", [P, NT * 2 * K], F16, kind="ExternalInput")
    gcn = nc.dram_tensor("gcn", [BPC, C, K], F16, kind="ExternalInput")
    # merged consts: [w2 (NCLS*C) | b2 (NCLS)] broadcast over partitions
    wb = nc.dram_tensor("wb", [P, NCLS * C + NCLS], F32, kind="ExternalInput")
    # [p, t, j] layout; host transposes to [t*128+p, j]
    out = nc.dram_tensor("out", [P, NT * NCLS], F32, kind="ExternalOutput")

    mult = mybir.AluOpType.mult
    add = mybir.AluOpType.add

    with tile.TileContext(nc) as tc:
        with (
            tc.tile_pool(name="gcnp", bufs=6) as gcnp,
            tc.tile_pool(name="actp", bufs=6) as actp,
            tc.tile_pool(name="big", bufs=1) as big,
        ):
            feat = big.tile([P, NT * C], F32)
            dscr = big.tile([P, K], F16)   # discard target for STT full out
            wb_sb = big.tile([P, NCLS * C + NCLS], F32)
            w2_sb = wb_sb[:, :NCLS * C].rearrange("p (j c) -> p j c", c=C)
            b2_sb = wb_sb[:, NCLS * C:]
            y = big.tile([P, NT * NCLS], F32)
            g0a = big.tile([P, 3 * K], F16)    # tile-0 gcn rows c0..2
            g0b = big.tile([P, 6 * K], F16)    # tile-0 gcn rows c3..8
            g15a = big.tile([P, 6 * K], F16)   # tile-15 gcn rows c0..5
            g15b = big.tile([P, 3 * K], F16)   # tile-15 gcn rows c6..8

            def ln_proj(t0, ntl):
                """Batched LayerNorm + projection for tiles [t0, t0+ntl)."""
                f3 = feat[:, t0 * C:(t0 + ntl) * C].rearrange(
                    "p (t c) -> p t c", c=C
                )
                negmu = big.tile([P, ntl], F32, tag=f"negmu{t0}")
                nc.vector.tensor_reduce(
                    out=negmu[:], in_=f3, axis=mybir.AxisListType.X, op=add
                )
                nc.vector.tensor_scalar_mul(negmu[:], negmu[:], -1.0 / C)
                cent = big.tile([P, ntl * C], F32, tag=f"cent{t0}")
                c3 = cent[:].rearrange("p (t c) -> p t c", c=C)
                nc.vector.tensor_tensor(
                    c3, f3, negmu[:][:, :, None].to_broadcast([P, ntl, C]), op=add
                )
                sq = big.tile([P, ntl * C], F32, tag=f"sq{t0}")
                s3 = sq[:].rearrange("p (t c) -> p t c", c=C)
                nc.vector.tensor_tensor(s3, c3, c3, op=mult)
                varp = big.tile([P, ntl], F32, tag=f"varp{t0}")
                nc.vector.tensor_reduce(
                    out=varp[:], in_=s3, axis=mybir.AxisListType.X, op=add
                )
                nc.vector.tensor_scalar(
                    out=varp[:], in0=varp[:], scalar1=1.0 / C, scalar2=LN_EPS,
                    op0=mult, op1=add,
                )
                std = big.tile([P, ntl], F32, tag=f"std{t0}")
                nc.scalar.activation(
                    std[:], varp[:], mybir.ActivationFunctionType.Sqrt
                )
                rstd = big.tile([P, ntl], F32, tag=f"rstd{t0}")
                nc.vector.reciprocal(rstd[:], std[:])
                xhat = big.tile([P, ntl * C], F32, tag=f"xhat{t0}")
                x3 = xhat[:].rearrange("p (t c) -> p t c", c=C)
                nc.vector.tensor_tensor(
                    x3, c3, rstd[:][:, :, None].to_broadcast([P, ntl, C]), op=mult
                )
                prod = big.tile([P, ntl * NCLS * C], F32, tag=f"prod{t0}")
                p4 = prod[:].rearrange("p (t j c) -> p t j c", j=NCLS, c=C)
                nc.vector.tensor_tensor(
                    p4,
                    x3[:, :, None, :].to_broadcast([P, ntl, NCLS, C]),
                    w2_sb[:, None, :, :].to_broadcast([P, ntl, NCLS, C]),
                    op=mult,
                )
                y3 = y[:, t0 * NCLS:(t0 + ntl) * NCLS].rearrange(
                    "p (t j) -> p t j", j=NCLS
                )
                nc.vector.tensor_reduce(
                    out=y3, in_=p4, axis=mybir.AxisListType.X, op=add
                )
                nc.vector.tensor_tensor(
                    y3, y3, b2_sb[:, None, :].to_broadcast([P, ntl, NCLS]), op=add
                )

            def dot(accum_col, g_ap, a_ap):
                """One fp16 dot product on DVE, fp32 accumulate."""
                nc.vector.scalar_tensor_tensor(
                    out=dscr[:],
                    in0=g_ap,
                    scalar=1.0,
                    in1=a_ap,
                    op0=mult,
                    op1=mult,
                    accum_out=feat[:, accum_col:accum_col + 1],
                )

            # ---- tile 0: split gcn 3/3/3 so first dots ungate early ----
            a0 = actp.tile([P, 2 * K], F16, tag="a")
            nc.sync.dma_start(a0[:], act[:, 0:2 * K])
            g0 = gcn[0:P]  # [128, 9, 512]
            nc.sync.dma_start(
                g0a[:].rearrange("p (c k) -> p c k", c=3), g0[:, 0:3, :]
            )
            nc.sync.dma_start(
                g0b[:, :3 * K].rearrange("p (c k) -> p c k", c=3), g0[:, 3:6, :]
            )
            nc.sync.dma_start(
                g0b[:, 3 * K:].rearrange("p (c k) -> p c k", c=3), g0[:, 6:9, :]
            )
            nc.scalar.dma_start(wb_sb[:], wb[:])
            ae, am = a0[:, 0:K], a0[:, K:2 * K]
            for c in range(3):
                dot(c, g0a[:, c * K:(c + 1) * K], ae)
            for c in range(3, 9):
                dot(c, g0b[:, (c - 3) * K:(c - 2) * K], am)

            # ---- tiles 1..14: streaming steady state ----
            for t in range(1, NT - 1):
                a_t = actp.tile([P, 2 * K], F16, tag="a")
                nc.scalar.dma_start(a_t[:], act[:, t * 2 * K:(t + 1) * 2 * K])
                g_t = gcnp.tile([P, C * K], F16)
                nc.sync.dma_start(
                    g_t[:].rearrange("p (c k) -> p c k", c=C),
                    gcn[t * P:(t + 1) * P],
                )
                ae, am = a_t[:, 0:K], a_t[:, K:2 * K]
                for c in range(C):
                    dot(t * C + c, g_t[:, c * K:(c + 1) * K], ae if c < 3 else am)
                # LN chunks slot into DVE slack (DMA has 2x headroom in fp16)
                if t == 5:
                    ln_proj(0, 4)
                elif t == 9:
                    ln_proj(4, 4)
                elif t == 13:
                    ln_proj(8, 4)
                elif t == 14:
                    ln_proj(12, 3)

            # ---- tile 15: split 6/3 to shorten the drain ----
            t = NT - 1
            a15 = actp.tile([P, 2 * K], F16, tag="a")
            nc.scalar.dma_start(a15[:], act[:, t * 2 * K:(t + 1) * 2 * K])
            g15 = gcn[t * P:(t + 1) * P]
            nc.sync.dma_start(
                g15a[:].rearrange("p (c k) -> p c k", c=6), g15[:, 0:6, :]
            )
            nc.sync.dma_start(
                g15b[:].rearrange("p (c k) -> p c k", c=3), g15[:, 6:9, :]
            )
            # first output piece: y[0:12] is final after ln_proj(8, 4)
            nc.scalar.dma_start(out[:, :12 * NCLS], y[:, :12 * NCLS])
            ae, am = a15[:, 0:K], a15[:, K:2 * K]
            for c in range(6):
                dot(t * C + c, g15a[:, c * K:(c + 1) * K], ae if c < 3 else am)
            for c in range(6, 9):
                dot(t * C + c, g15b[:, (c - 6) * K:(c - 5) * K], am)
            ln_proj(15, 1)
            nc.scalar.dma_start(out[:, 12 * NCLS:], y[:, 12 * NCLS:])

    nc.finalize()
    return nc


def _get_nc():
    global _NC
    if _NC is None:
        _NC = _build_nc()
    return _NC


def _run(inputs, **spmd_kwargs):
    eyebrow = np.asarray(inputs["eyebrow"]).astype(np.float16)
    mouth = np.asarray(inputs["mouth"]).astype(np.float16)
    gcn = np.ascontiguousarray(np.asarray(inputs["gcn"]).astype(np.float16))
    ln_w = np.asarray(inputs["ln_weight"], dtype=np.float32)
    ln_b = np.asarray(inputs["ln_bias"], dtype=np.float32)
    lin_w = np.asarray(inputs["lin_weight"], dtype=np.float32)
    lin_b = np.asarray(inputs["lin_bias"], dtype=np.float32)

    # Fold LN affine + Linear: normed*ln_w + ln_b then @ lin_w.T + lin_b
    #   == xhat @ W2 + b2 with W2[c,j] = ln_w[c]*lin_w[j,c], b2 = lin_w@ln_b + lin_b
    w2 = (lin_w * ln_w[None, :]).astype(np.float32)        # [NCLS, C] = W2.T
    b2 = (lin_w @ ln_b + lin_b).astype(np.float32)         # [NCLS]
    wb1 = np.concatenate([w2.ravel(), b2]).astype(np.float32)
    wb = np.ascontiguousarray(np.broadcast_to(wb1[None], (P, NCLS * C + NCLS)))

    # per-core partition-major interleaved act layout: [P, NT, 2, K]
    a_sh = np.stack(
        [eyebrow.reshape(N_CORES, NT, P, K), mouth.reshape(N_CORES, NT, P, K)],
        axis=3,
    )  # [cores, NT, P, 2, K]
    a_sh = np.ascontiguousarray(a_sh.transpose(0, 2, 1, 3, 4)).reshape(
        N_CORES, P, NT * 2 * K
    )
    g_sh = gcn.reshape(N_CORES, BPC, C, K)
    in_maps = [
        {"act": a_sh[c], "gcn": g_sh[c], "wb": wb}
        for c in range(N_CORES)
    ]

    res = run_bass_kernel_spmd(
        _get_nc(), in_maps, core_ids=list(range(N_CORES)), **spmd_kwargs
    )
    # out[p, t*5+j] -> full[(core, t*128+p), j]
    out = np.concatenate(
        [
            r["out"].reshape(P, NT, NCLS).transpose(1, 0, 2).reshape(BPC, NCLS)
            for r in res.results
        ],
        axis=0,
    )
    return out, res


def kernel(**inputs):
    out, _ = _run(inputs)
    return out


# revision 12
# speedup vs baseline: 1.1468x; 1.0010x over previous
"""Trainium2 Bass kernel for nn_AUFusion (dense_mlp, memory-bound).

Reference computation (per sample b):
  feat[b, c]   = sum_k act_c[b, k] * gcn[b, c, k]    act_c = eyebrow (c<3) / mouth (c>=3)
  normed       = LayerNorm(feat) * ln_w + ln_b       (over the 9 features, eps=1e-6)
  out[b, :]    = normed @ lin_w.T + lin_b            (9 -> 5)

Strategy: pure data parallelism, batch 16384 -> 2048 per core on 8 cores.
On-chip layout puts 128 samples on partitions and K=512 on the free axis.

The 16 DMA queues sustain ~27 GB/s each (432 GB/s aggregate); in fp32 the 44
MiB/core input stream has a hard ~107 us floor and DVE (144 dot products at
~0.69 us) needs ~105 us — both saturated. The host therefore downcasts the
streamed tensors to fp16 (inputs are ~N(0,1); the fp32 accumulation keeps
rel err ~1e-4, well under the 2e-2 gate), halving the stream to ~53 us and
making compute the only critical path. Each dot product is one fp16
scalar_tensor_tensor on DVE (out = (g*1)*a with fp32 accum_out, ~0.61 us);
LayerNorm + the (host-folded) LN-affine+Linear projection run batched over
4-tile chunks in DVE slack. Tile 0's gcn is split 3/3/3 so the first dots
ungate early; tile 15 is split 6/3 to shorten the drain. Output is stored
as [128, 16, 5] and transposed on host.
"""

import numpy as np

import concourse.bacc as bacc
import concourse.tile as tile
from concourse import mybir
from concourse.bass_utils import run_bass_kernel_spmd

N_CORES = 8
B = 16384
BPC = B // N_CORES          # samples per core
K = 512
C = 9                       # in features
NCLS = 5                    # num classes
P = 128                     # partitions
NT = BPC // P               # 16 sample-tiles per core
LN_EPS = 1e-6
F32 = mybir.dt.float32
F16 = mybir.dt.float16

_NC = None  # built once, reused across calls


def _build_nc():
    nc = bacc.Bacc(None)
    # host-pretransposed, interleaved act (fp16): a[p, t*2K + (0:K)] =
    # eyebrow[t*128+p, :], a[p, t*2K + (K:2K)] = mouth[t*128+p, :]
    act = nc.dram_tensor("act", [P, NT * 2 * K], F16, kind="ExternalInput")
    gcn = nc.dram_tensor("gcn", [BPC, C, K], F16, kind="ExternalInput")
    # merged consts: [w2 (NCLS*C) | b2 (NCLS)] broadcast over partitions
    wb = nc.dram_tensor("wb", [P, NCLS * C + NCLS], F32, kind="ExternalInput")
    # [p, t, j] layout; host transposes to [t*128+p, j]
    out = nc.dram_tensor("out", [P, NT * NCLS], F32, kind="ExternalOutput")

    mult = mybir.AluOpType.mult
    add = mybir.AluOpType.add

    with tile.TileContext(nc) as tc:
        with (
            tc.tile_pool(name="gcnp", bufs=6) as gcnp,
            tc.tile_pool(name="actp", bufs=6) as actp,
            tc.tile_pool(name="big", bufs=1) as big,
        ):
            feat = big.tile([P, NT * C], F32)
            dscr = big.tile([P, K], F16)   # discard target for STT full out
            wb_sb = big.tile([P, NCLS * C + NCLS], F32)
            w2_sb = wb_sb[:, :NCLS * C].rearrange("p (j c) -> p j c", c=C)
            b2_sb = wb_sb[:, NCLS * C:]
            y = big.tile([P, NT * NCLS], F32)
            g0a = big.tile([P, 3 * K], F16)    # tile-0 gcn rows c0..2
            g0b = big.tile([P, 6 * K], F16)    # tile-0 gcn rows c3..8
            g15a = big.tile([P, 6 * K], F16)   # tile-15 gcn rows c0..5
            g15b = big.tile([P, 3 * K], F16)   # tile-15 gcn rows c6..8

            def ln_proj(t0, ntl):
                """Batched LayerNorm + projection for tiles [t0, t0+ntl)."""
                f3 = feat[:, t0 * C:(t0 + ntl) * C].rearrange(
                    "p (t c) -> p t c", c=C
                )
                negmu = big.tile([P, ntl], F32, tag=f"negmu{t0}")
                nc.vector.tensor_reduce(
                    out=negmu[:], in_=f3, axis=mybir.AxisListType.X, op=add
                )
                nc.vector.tensor_scalar_mul(negmu[:], negmu[:], -1.0 / C)
                cent = big.tile([P, ntl * C], F32, tag=f"cent{t0}")
                c3 = cent[:].rearrange("p (t c) -> p t c", c=C)
                nc.vector.tensor_tensor(
                    c3, f3, negmu[:][:, :, None].to_broadcast([P, ntl, C]), op=add
                )
                sq = big.tile([P, ntl * C], F32, tag=f"sq{t0}")
                s3 = sq[:].rearrange("p (t c) -> p t c", c=C)
                nc.vector.tensor_tensor(s3, c3, c3, op=mult)
                varp = big.tile([P, ntl], F32, tag=f"varp{t0}")
                nc.vector.tensor_reduce(
                    out=varp[:], in_=s3, axis=mybir.AxisListType.X, op=add
                )
                nc.vector.tensor_scalar(
                    out=varp[:], in0=varp[:], scalar1=1.0 / C, scalar2=LN_EPS,
                    op0=mult, op1=add,
                )
                std = big.tile([P, ntl], F32, tag=f"std{t0}")
                nc.scalar.activation(
                    std[:], varp[:], mybir.ActivationFunctionType.Sqrt
                )
                rstd = big.tile([P, ntl], F32, tag=f"rstd{t0}")
                nc.vector.reciprocal(rstd[:], std[:])
                xhat = big.tile([P, ntl * C], F32, tag=f"xhat{t0}")
                x3 = xhat[:].rearrange("p (t c) -> p t c", c=C)
                nc.vector.tensor_tensor(
                    x3, c3, rstd[:][:, :, None].to_broadcast([P, ntl, C]), op=mult
                )
                prod = big.tile([P, ntl * NCLS * C], F32, tag=f"prod{t0}")
                p4 = prod[:].rearrange("p (t j c) -> p t j c", j=NCLS, c=C)
                nc.vector.tensor_tensor(
                    p4,
                    x3[:, :, None, :].to_broadcast([P, ntl, NCLS, C]),
                    w2_sb[:, None, :, :].to_broadcast([P, ntl, NCLS, C]),
                    op=mult,
                )
                y3 = y[:, t0 * NCLS:(t0 + ntl) * NCLS].rearrange(
                    "p (t j) -> p t j", j=NCLS
                )
                nc.vector.tensor_reduce(
                    out=y3, in_=p4, axis=mybir.AxisListType.X, op=add
                )
                nc.vector.tensor_tensor(
                    y3, y3, b2_sb[:, None, :].to_broadcast([P, ntl, NCLS]), op=add
                )

            def dot(accum_col, g_ap, a_ap):
                """One fp16 dot product on DVE, fp32 accumulate."""
                nc.vector.scalar_tensor_tensor(
                    out=dscr[:],
                    in0=g_ap,
                    scalar=1.0,
                    in1=a_ap,
                    op0=mult,
                    op1=mult,
                    accum_out=feat[:, accum_col:accum_col + 1],
                )

            # ---- tile 0: split gcn 3/3/3 so first dots ungate early ----
            a0 = actp.tile([P, 2 * K], F16, tag="a")
            nc.sync.dma_start(a0[:, 0:K], act[:, 0:K])
            g0 = gcn[0:P]  # [128, 9, 512]
            nc.sync.dma_start(
                g0a[:, 0:K].rearrange("p (c k) -> p c k", c=1), g0[:, 0:1, :]
            )
            nc.sync.dma_start(a0[:, K:2 * K], act[:, K:2 * K])
            nc.sync.dma_start(
                g0a[:, K:].rearrange("p (c k) -> p c k", c=2), g0[:, 1:3, :]
            )
            nc.sync.dma_start(
                g0b[:, :3 * K].rearrange("p (c k) -> p c k", c=3), g0[:, 3:6, :]
            )
            nc.sync.dma_start(
                g0b[:, 3 * K:].rearrange("p (c k) -> p c k", c=3), g0[:, 6:9, :]
            )
            nc.scalar.dma_start(wb_sb[:], wb[:])
            ae, am = a0[:, 0:K], a0[:, K:2 * K]
            for c in range(3):
                dot(c, g0a[:, c * K:(c + 1) * K], ae)
            for c in range(3, 9):
                dot(c, g0b[:, (c - 3) * K:(c - 2) * K], am)

            # ---- tiles 1..14: streaming steady state ----
            for t in range(1, NT - 1):
                a_t = actp.tile([P, 2 * K], F16, tag="a")
                nc.scalar.dma_start(a_t[:], act[:, t * 2 * K:(t + 1) * 2 * K])
                g_t = gcnp.tile([P, C * K], F16)
                nc.sync.dma_start(
                    g_t[:].rearrange("p (c k) -> p c k", c=C),
                    gcn[t * P:(t + 1) * P],
                )
                ae, am = a_t[:, 0:K], a_t[:, K:2 * K]
                for c in range(C):
                    dot(t * C + c, g_t[:, c * K:(c + 1) * K], ae if c < 3 else am)
                # LN chunks slot into DVE slack (DMA has 2x headroom in fp16)
                if t == 5:
                    ln_proj(0, 4)
                elif t == 9:
                    ln_proj(4, 4)
                elif t == 13:
                    ln_proj(8, 4)
                elif t == 14:
                    ln_proj(12, 3)

            # ---- tile 15: split 6/3 to shorten the drain ----
            t = NT - 1
            a15 = actp.tile([P, 2 * K], F16, tag="a")
            nc.scalar.dma_start(a15[:], act[:, t * 2 * K:(t + 1) * 2 * K])
            g15 = gcn[t * P:(t + 1) * P]
            nc.sync.dma_start(
                g15a[:].rearrange("p (c k) -> p c k", c=6), g15[:, 0:6, :]
            )
            nc.sync.dma_start(
                g15b[:].rearrange("p (c k) -> p c k", c=3), g15[:, 6:9, :]
            )
            # first output piece: y[0:12] is final after ln_proj(8, 4)
            nc.scalar.dma_start(out[:, :12 * NCLS], y[:, :12 * NCLS])
            ae, am = a15[:, 0:K], a15[:, K:2 * K]
            for c in range(6):
                dot(t * C + c, g15a[:, c * K:(c + 1) * K], ae if c < 3 else am)
            for c in range(6, 9):
                dot(t * C + c, g15b[:, (c - 6) * K:(c - 5) * K], am)
            ln_proj(15, 1)
            nc.scalar.dma_start(out[:, 12 * NCLS:], y[:, 12 * NCLS:])

    nc.finalize()
    return nc


def _get_nc():
    global _NC
    if _NC is None:
        _NC = _build_nc()
    return _NC


def _run(inputs, **spmd_kwargs):
    eyebrow = np.asarray(inputs["eyebrow"]).astype(np.float16)
    mouth = np.asarray(inputs["mouth"]).astype(np.float16)
    gcn = np.ascontiguousarray(np.asarray(inputs["gcn"]).astype(np.float16))
    ln_w = np.asarray(inputs["ln_weight"], dtype=np.float32)
    ln_b = np.asarray(inputs["ln_bias"], dtype=np.float32)
    lin_w = np.asarray(inputs["lin_weight"], dtype=np.float32)
    lin_b = np.asarray(inputs["lin_bias"], dtype=np.float32)

    # Fold LN affine + Linear: normed*ln_w + ln_b then @ lin_w.T + lin_b
    #   == xhat @ W2 + b2 with W2[c,j] = ln_w[c]*lin_w[j,c], b2 = lin_w@ln_b + lin_b
    w2 = (lin_w * ln_w[None, :]).astype(np.float32)        # [NCLS, C] = W2.T
    b2 = (lin_w @ ln_b + lin_b).astype(np.float32)         # [NCLS]
    wb1 = np.concatenate([w2.ravel(), b2]).astype(np.float32)
    wb = np.ascontiguousarray(np.broadcast_to(wb1[None], (P, NCLS * C + NCLS)))

    # per-core partition-major interleaved act layout: [P, NT, 2, K]
    a_sh = np.stack(
        [eyebrow.reshape(N_CORES, NT, P, K), mouth.reshape(N_CORES, NT, P, K)],
        axis=3,
    )  # [cores, NT, P, 2, K]
    a_sh = np.ascontiguousarray(a_sh.transpose(0, 2, 1, 3, 4)).reshape(
        N_CORES, P, NT * 2 * K
    )
    g_sh = gcn.reshape(N_CORES, BPC, C, K)
    in_maps = [
        {"act": a_sh[c], "gcn": g_sh[c], "wb": wb}
        for c in range(N_CORES)
    ]

    res = run_bass_kernel_spmd(
        _get_nc(), in_maps, core_ids=list(range(N_CORES)), **spmd_kwargs
    )
    # out[p, t*5+j] -> full[(core, t*128+p), j]
    out = np.concatenate(
        [
            r["out"].reshape(P, NT, NCLS).transpose(1, 0, 2).reshape(BPC, NCLS)
            for r in res.results
        ],
        axis=0,
    )
    return out, res


def kernel(**inputs):
    out, _ = _run(inputs)
    return out
